# revision 15
# baseline (speedup 1.0000x reference)
"""Trainium2 Bass kernel for nn_DSSM (Mamba-like selective-scan block).

Reference math (B=4, L=4096, D=1024, ED=2048, N=16, K=3):
    proj = x @ W_in.T ; x_conv_pre, x_ssm = split(proj)
    x_conv = depthwise_conv1d(x_conv_pre, conv_w, pad=1)
    dt = mean_e(x_ssm); dtv = dt * W_dt[:,0]
    a = dtv @ A ; u = (dtv * x_ssm) @ Bm          # [b, l, N]
    m_t = a_t * m_{t-1} + u_t  (scan over l)
    y = m @ Cm + Dv * x_ssm
    z = x_conv * sig(y) + y * (1 - sig(y))
    out = z @ W_out.T + x

Algebraic folding (host, exact):
    dt = x @ w_mean              where w_mean = mean_e(W_ssm)
    a  = dt * s_a                where s_a = A.T @ W_dt[:,0]
    u  = dt * (x @ G)            where G = W_ssm.T @ (W_dt[:,0] * Bm)
    Dv folded into the ssm half of W_in (y = m@Cm + x@(Dv*W_ssm).T)

Sharding: core c -> batch c//2, L-half c%2 (2048 rows each). The scan is
seeded by a 128-row warmup for odd cores (max |a| = 0.54 empirically, so
the recurrence forgets its initial state within ~50 steps). Conv boundary
columns come from a small standalone matmul pass (psum [e, 8] layout).

V4: ssm-half / out-proj GEMMs in bf16; conv-half GEMM in fp8e4m3 with
DoubleRow perf mode (2 contraction rows per PE pass; measured end-to-end
rel err 4.5e-3 vs the 2e-2 gate). W_in / W_out / conv fp8 weights stay
resident in SBUF across reps (first-rep loads spread over all three DMA
queues in consumption order); out_proj computed transposed
(out^T[d, r] = sum_e WO[e,d]^T z[e,r]) so the residual comes from the
already-loaded xT tiles and the output stores as [D, RPC] (host
transposes back). Scan path is ONE 48-wide GEMM: partitions 0-15 get
x@G, partitions 32-47 get dt broadcast to all 16 rows directly (the
stationary is w_mean replicated 16x; partition offsets must be 32-
aligned for PSUM reads, hence the pad). x ships twice: bf16 x^T
(ssm/scan/residual) and fp8 DoubleRow layout (conv). Steady-state HBM
traffic per rep: 3.3MB in + 8.4MB out; PE ~96% busy in TimelineSim
(~270us/rep steady, bf16 tensor floor ~250us).
"""
import sys
sys.path.insert(0, '/opt/trn_rl_repo')

import numpy as np
import ml_dtypes

import concourse.bass as bass
import concourse.bacc as bacc
import concourse.tile as tile
import concourse.mybir as mybir
from concourse.bass_utils import run_bass_kernel_spmd

F32 = mybir.dt.float32
F32R = mybir.dt.float32r
BF16 = mybir.dt.bfloat16
FP8 = mybir.dt.float8e4
DR = mybir.MatmulPerfMode.DoubleRow
MULT = mybir.AluOpType.mult
ADD = mybir.AluOpType.add
SUBT = mybir.AluOpType.subtract
SIG = mybir.ActivationFunctionType.Sigmoid

B_SZ, L, D, ED, N = 4, 4096, 1024, 2048, 16
N_CORES = 8
RPC = 2048          # rows per core
SUB = 512           # rows per sub-chunk
NSUB = RPC // SUB   # 4
WARM = 128          # scan warmup rows (max |a| = 0.54 -> leak ~1e-34)
NKT = D // 128      # 8 k-tiles over the contraction dim
NET = ED // 128     # 16 e-tiles per half
NJP = D // 256      # 4 DoubleRow passes over the contraction dim

# conv halo row indices relative to the core's first row: head/tail of each
# sub-chunk boundary. head(s) = HALO_HEAD[s], tail(s) = HALO_TAIL[s].
HALO_REL = [-1, 511, 512, 1023, 1024, 1535, 1536, 2048]
HALO_HEAD = [0, 1, 3, 5]
HALO_TAIL = [2, 4, 6, 7]

_CACHED_NC = None


def build_kernel(reps=1):
    nc = bacc.Bacc("TRN2", target_bir_lowering=False, debug=False,
                   num_devices=N_CORES)

    XT = nc.dram_tensor("xt", [D, RPC], BF16, kind="ExternalInput")
    XQ = nc.dram_tensor("xq", [128, 2 * NJP * RPC], FP8, kind="ExternalInput")
    XWT = nc.dram_tensor("xwt", [D, WARM], BF16, kind="ExternalInput")
    XH8 = nc.dram_tensor("xh8", [128, 2 * NJP * 8], FP8, kind="ExternalInput")
    WTS = nc.dram_tensor("wts", [D, ED], BF16, kind="ExternalInput")
    WC8 = nc.dram_tensor("wc8", [128, 2 * NJP * ED], FP8, kind="ExternalInput")
    WO = nc.dram_tensor("wo", [ED, D], BF16, kind="ExternalInput")
    CM = nc.dram_tensor("cm", [N, ED], F32R, kind="ExternalInput")
    HM = nc.dram_tensor("hm", [D, 48], BF16, kind="ExternalInput")
    SA = nc.dram_tensor("sa", [N, 1], F32, kind="ExternalInput")
    CW = nc.dram_tensor("cw", [NET, 128, 3], F32, kind="ExternalInput")
    OUT = nc.dram_tensor("out", [D, RPC], F32, kind="ExternalOutput")

    with tile.TileContext(nc) as tc:
        with (
            tc.tile_pool(name="const", bufs=1) as cpool,
            tc.tile_pool(name="xt", bufs=3) as xt_pool,
            tc.tile_pool(name="pre", bufs=3) as pre_pool,
            tc.tile_pool(name="gy", bufs=3) as gy_pool,
            tc.tile_pool(name="cvt", bufs=2) as cv_pool,
            tc.tile_pool(name="zp", bufs=32) as z_pool,
            tc.tile_pool(name="scn", bufs=1) as s_pool,
            tc.tile_pool(name="ob", bufs=2) as o_pool,
            tc.tile_pool(name="sps", bufs=2, space="PSUM") as s_ps,
            tc.tile_pool(name="fps", bufs=4, space="PSUM") as f_ps,
            tc.tile_pool(name="ops", bufs=2, space="PSUM") as o_ps,
        ):
            # ---- resident constants ----
            h_sb = cpool.tile([128, NKT * 48], BF16, tag="hm")
            nc.sync.dma_start(
                h_sb[:].rearrange("p (k j) -> p k j", k=NKT),
                HM[:].rearrange("(k p) j -> p k j", p=128))
            sa_sb = cpool.tile([N, 1], F32, tag="sa")
            nc.sync.dma_start(sa_sb[:], SA[:])
            halo_all = cpool.tile([128, NET * 8], F32, tag="halo")
            wc8_sb = cpool.tile([128, 2 * NJP * ED], FP8, tag="wc8")
            wt_sb = cpool.tile([128, NKT * ED], BF16, tag="wt")
            wo_sb = cpool.tile([128, NET * D], BF16, tag="wo")
            cm_sb = cpool.tile([N, ED], F32R, tag="cm")
            cw_sb = cpool.tile([128, NET * 3], F32, tag="cw")
            xh8_sb = cpool.tile([128, 2 * NJP * 8], FP8, tag="xh8")
            zero16 = cpool.tile([N, 1], F32, tag="zero16")
            nc.vector.memset(zero16[:], 0.0)

            def wts_strip(k, i):
                """[128, 128] bf16 stationary strip for ssm e-tile i."""
                c0 = k * ED + i * 128
                return wt_sb[:, c0:c0 + 128]

            def wc8_strip(j, i):
                """[128, 2, 128] fp8 DoubleRow stationary: pass j, e-tile i."""
                base = wc8_sb[:, j * 2 * ED:(j + 1) * 2 * ED]
                return base.rearrange("p (t e) -> p t e",
                                      t=2)[:, :, i * 128:(i + 1) * 128]

            def xq_pass(xq_t, j, width=SUB):
                """[128, 2, width] fp8 DoubleRow moving slice for pass j."""
                base = xq_t[:, j * 2 * width:(j + 1) * 2 * width]
                return base.rearrange("p (t c) -> p t c", t=2)

            prev_m = [None, 0]   # tile, width

            def scan_path(xt_t, first, width=SUB):
                """two 16-wide GEMMs (x@G and broadcast dt) -> a,u -> scan.

                The dt broadcast rides the GEMM: stationary w_mean (x) ones16
                makes pdtb[n, c] = dt[c] for all n directly."""
                pall = s_ps.tile([48, width], F32, tag="sps")
                for k in range(NKT):
                    nc.tensor.matmul(pall[:], h_sb[:, k * 48:(k + 1) * 48],
                                     xt_t[:, k * width:(k + 1) * width],
                                     start=(k == 0), stop=(k == NKT - 1))
                pdtb = pall[32:48, :]
                sv = s_pool.tile([N, width], F32, tag="sv")
                nc.vector.tensor_copy(sv[:], pall[0:N, :])
                a_sb = s_pool.tile([N, width], F32, tag="a")
                nc.vector.tensor_scalar_mul(a_sb[:], pdtb, sa_sb[:])
                u_sb = s_pool.tile([N, width], F32, tag="u")
                nc.vector.tensor_mul(u_sb[:], sv[:], pdtb)
                m = s_pool.tile([N, width], F32, tag="m", bufs=2)
                if first:
                    init = zero16[:]
                else:
                    pm, pw = prev_m
                    init = pm[:, pw - 1:pw]
                nc.vector.tensor_tensor_scan(m[:], a_sb[:], u_sb[:], init,
                                             op0=MULT, op1=ADD)
                prev_m[0] = m
                prev_m[1] = width
                return m

            def load_xt(s):
                """One batched bf16 x^T tile [128, (k c)] for sub s."""
                xt_t = xt_pool.tile([128, NKT * SUB], BF16, tag="xt")
                nc.sync.dma_start(
                    xt_t[:].rearrange("p (k c) -> p k c", k=NKT),
                    XT[:].rearrange("(k p) c -> p k c", p=128)
                    [:, :, s * SUB:(s + 1) * SUB])
                return xt_t

            def load_xq(s):
                """One batched fp8 DoubleRow tile [128, (j t c)] for sub s."""
                xq_t = xt_pool.tile([128, 2 * NJP * SUB], FP8, tag="xq")
                nc.sync.dma_start(
                    xq_t[:].rearrange("p (j c) -> p j c", j=2 * NJP),
                    XQ[:].rearrange("p (j c) -> p j c", j=2 * NJP)
                    [:, :, s * SUB:(s + 1) * SUB])
                return xq_t

            def in_proj_sub(s, xt_t, xq_t, first_rep):
                """in-proj + conv + gate for sub s -> list of 16 z tiles."""
                m = scan_path(xt_t, first=False)
                m_bf = s_pool.tile([N, SUB], F32R, tag="mbf")
                nc.scalar.copy(m_bf[:], m[:])

                z_tiles = []
                for i in range(NET):
                    # conv half e-tile: fp8 DoubleRow (4 passes over K)
                    pc = f_ps.tile([128, SUB], F32, tag="fps")
                    for j in range(NJP):
                        nc.tensor.matmul(pc[:], wc8_strip(j, i),
                                         xq_pass(xq_t, j), start=(j == 0),
                                         stop=(j == NJP - 1), perf_mode=DR)
                    if s == 0:
                        # conv halo rows: one standalone pass per e-tile
                        ph = s_ps.tile([128, 8], F32, tag="sps")
                        for j in range(NJP):
                            nc.tensor.matmul(ph[:], wc8_strip(j, i),
                                             xq_pass(xh8_sb, j, width=8),
                                             start=(j == 0),
                                             stop=(j == NJP - 1), perf_mode=DR)
                        nc.vector.tensor_copy(
                            halo_all[:, i * 8:(i + 1) * 8], ph[:])
                    pre = pre_pool.tile([128, SUB + 2], F32, tag="pre")
                    nc.scalar.copy(pre[:, 1:SUB + 1], pc[:])
                    hc = i * 8 + HALO_HEAD[s]
                    tc_ = i * 8 + HALO_TAIL[s]
                    nc.vector.tensor_copy(pre[:, 0:1], halo_all[:, hc:hc + 1])
                    nc.vector.tensor_copy(pre[:, SUB + 1:SUB + 2],
                                          halo_all[:, tc_:tc_ + 1])
                    # ssm half e-tile (+ y accumulation), bf16
                    py = f_ps.tile([128, SUB], F32, tag="fps")
                    for k in range(NKT):
                        nc.tensor.matmul(py[:], wts_strip(k, i),
                                         xt_t[:, k * SUB:(k + 1) * SUB],
                                         start=(k == 0), stop=False)
                    nc.tensor.matmul(py[:], cm_sb[:, i * 128:(i + 1) * 128],
                                     m_bf[:], start=False, stop=True)
                    g = gy_pool.tile([128, SUB], F32, tag="g")
                    nc.scalar.activation(g[:], py[:], SIG)
                    ysb = gy_pool.tile([128, SUB], F32, tag="ysb")
                    nc.scalar.copy(ysb[:], py[:])
                    # conv + gate: w = conv(pre) - y ; z = y + sig(y)*w
                    w0 = cw_sb[:, i * 3 + 0:i * 3 + 1]
                    w1 = cw_sb[:, i * 3 + 1:i * 3 + 2]
                    w2 = cw_sb[:, i * 3 + 2:i * 3 + 3]
                    s1 = cv_pool.tile([128, SUB], F32, tag="s1", bufs=1)
                    nc.vector.scalar_tensor_tensor(
                        s1[:], pre[:, 1:SUB + 1], w1, ysb[:],
                        op0=MULT, op1=SUBT)
                    s2 = cv_pool.tile([128, SUB], F32, tag="s2", bufs=1)
                    nc.vector.scalar_tensor_tensor(
                        s2[:], pre[:, 0:SUB], w0, s1[:], op0=MULT, op1=ADD)
                    wc = cv_pool.tile([128, SUB], F32, tag="wc")
                    nc.vector.scalar_tensor_tensor(
                        wc[:], pre[:, 2:SUB + 2], w2, s2[:], op0=MULT, op1=ADD)
                    t_ = cv_pool.tile([128, SUB], F32, tag="t", bufs=1)
                    nc.gpsimd.tensor_mul(t_[:], g[:], wc[:])
                    z = z_pool.tile([128, SUB], BF16, tag="z")
                    nc.gpsimd.tensor_add(z[:], t_[:], ysb[:])
                    z_tiles.append(z)
                return z_tiles

            def out_proj_sub(s, z_tiles, xt_t):
                """out^T[d, r] = sum_e WO[e,d]^T z[e,r]; residual from xT."""
                for dt_ in range(NKT):
                    po = o_ps.tile([128, SUB], F32, tag="ops")
                    for ei in range(NET):
                        nc.tensor.matmul(
                            po[:], wo_sb[:, ei * D + dt_ * 128:
                                         ei * D + (dt_ + 1) * 128],
                            z_tiles[ei][:], start=(ei == 0),
                            stop=(ei == NET - 1))
                    osb = o_pool.tile([128, SUB], F32, tag="osb")
                    nc.vector.tensor_add(osb[:], po[:],
                                         xt_t[:, dt_ * SUB:(dt_ + 1) * SUB])
                    nc.gpsimd.dma_start(
                        OUT[dt_ * 128:(dt_ + 1) * 128,
                            s * SUB:(s + 1) * SUB], osb[:])

            def emit_body(first_rep):
                # prime sub 0's loads first, then the warm scan
                xtw = xt_pool.tile([128, NKT * WARM], BF16, tag="xtw", bufs=1)
                nc.sync.dma_start(
                    xtw[:].rearrange("p (k c) -> p k c", k=NKT),
                    XWT[:].rearrange("(k p) c -> p k c", p=128))
                xt0 = load_xt(0)
                xq0 = load_xq(0)
                if first_rep:
                    nc.sync.dma_start(xh8_sb[:], XH8[:])
                    nc.sync.dma_start(cm_sb[:], CM[:])
                    nc.sync.dma_start(
                        cw_sb[:].rearrange("p (i k) -> p i k", i=NET),
                        CW[:].rearrange("i p k -> p i k"))
                    # weight loads spread over the three DMA queues in
                    # consumption order: conv fp8 + ssm e-tiles 8-15 on
                    # scalar, ssm e-tiles 0-7 on gpsimd, out-proj on sync
                    # (behind the small x loads, needed latest).
                    nc.scalar.dma_start(wc8_sb[:], WC8[:])
                    nc.gpsimd.dma_start(
                        wt_sb[:].rearrange("p (k e) -> p k e", k=NKT)
                        [:, :, 0:ED // 2],
                        WTS[:].rearrange("(k p) e -> p k e", p=128)
                        [:, :, 0:ED // 2])
                    nc.scalar.dma_start(
                        wt_sb[:].rearrange("p (k e) -> p k e", k=NKT)
                        [:, :, ED // 2:ED],
                        WTS[:].rearrange("(k p) e -> p k e", p=128)
                        [:, :, ED // 2:ED])
                    nc.sync.dma_start(
                        wo_sb[:].rearrange("p (i d) -> p i d", i=NET),
                        WO[:].rearrange("(i p) d -> p i d", p=128))
                scan_path(xtw, first=True, width=WARM)

                prev = None  # (s, z_tiles, xt_t) pending out-proj
                for s in range(NSUB):
                    if s == 0:
                        xt_t, xq_t = xt0, xq0
                    else:
                        xt_t, xq_t = load_xt(s), load_xq(s)
                    z_tiles = in_proj_sub(s, xt_t, xq_t, first_rep)
                    if prev is not None:
                        out_proj_sub(*prev)
                    prev = (s, z_tiles, xt_t)
                out_proj_sub(*prev)

            for rep in range(reps):
                emit_body(rep == 0)
    nc.compile()
    return nc


def prep_inputs(x, A, Bm, Cm, Dv, W_dt, conv_w, W_in, W_out):
    """Host-side folding + per-core sharding. Returns in_maps list."""
    x = np.asarray(x, np.float32)
    A = np.asarray(A, np.float32)
    Bm = np.asarray(Bm, np.float32)
    Cm = np.asarray(Cm, np.float32)
    Dv = np.asarray(Dv, np.float32)
    W_dt = np.asarray(W_dt, np.float32)
    conv_w = np.asarray(conv_w, np.float32)
    W_in = np.asarray(W_in, np.float32)
    W_out = np.asarray(W_out, np.float32)

    BF = ml_dtypes.bfloat16
    F8 = ml_dtypes.float8_e4m3

    def dr_pack(mat):
        """[D, C] -> DoubleRow layout [128, (j t C)] (j=pass, t=row slot)."""
        d, c = mat.shape
        return np.ascontiguousarray(
            mat.reshape(NJP, 2, 128, c).transpose(2, 0, 1, 3).reshape(
                128, NJP * 2 * c))

    W_conv = W_in[:ED]
    W_ssm = W_in[ED:]
    WTS = np.ascontiguousarray((W_ssm * Dv[:, None]).T).astype(BF)  # [D, ED]
    WC8 = dr_pack(np.ascontiguousarray(W_conv.T)).astype(F8)
    w_mean = W_ssm.mean(axis=0, dtype=np.float64).astype(np.float32)  # [D]
    G = (W_ssm.T.astype(np.float64) @ (W_dt[:, 0:1] * Bm).astype(np.float64)
         ).astype(np.float32)                                     # [D, N]
    HM = np.ascontiguousarray(
        np.concatenate([G, np.zeros((D, N), np.float32),
                        np.repeat(w_mean[:, None], N, axis=1)],
                       axis=1)).astype(BF)                        # [D, 48]
    s_a = (A.T.astype(np.float64) @ W_dt[:, 0].astype(np.float64)
           ).astype(np.float32)[:, None]                          # [N, 1]
    WO = np.ascontiguousarray(W_out.T).astype(BF)                 # [ED, D]
    CMb = np.ascontiguousarray(Cm)                                # [N, ED] f32r
    CW = np.ascontiguousarray(conv_w[:, 0, :].reshape(NET, 128, 3))

    x_flat = np.ascontiguousarray(x.reshape(B_SZ * L, D))
    in_maps = []
    for c in range(N_CORES):
        b, h = c // 2, c % 2
        g0 = b * L + h * RPC
        xs = x_flat[g0:g0 + RPC]
        xsT = np.ascontiguousarray(xs.T)                          # [D, RPC]
        if h == 1:
            xw = x_flat[g0 - WARM:g0]
        else:
            xw = np.zeros((WARM, D), np.float32)
        xh = np.zeros((8, D), np.float32)
        for j, rel in enumerate(HALO_REL):
            gr = g0 + rel
            if (h == 0 and rel < 0) or (h == 1 and rel >= RPC):
                continue  # out of batch -> zero pad
            xh[j] = x_flat[gr]
        in_maps.append({
            "xt": xsT.astype(BF),
            "xq": dr_pack(xsT).astype(F8),
            "xwt": np.ascontiguousarray(xw.T).astype(BF),
            "xh8": dr_pack(np.ascontiguousarray(xh.T)).astype(F8),
            "wts": WTS, "wc8": WC8, "wo": WO, "cm": CMb, "hm": HM,
            "sa": s_a, "cw": CW,
        })
    return in_maps


def kernel(**inputs):
    global _CACHED_NC
    if _CACHED_NC is None:
        _CACHED_NC = build_kernel()
    nc = _CACHED_NC
    in_maps = prep_inputs(**inputs)
    res = run_bass_kernel_spmd(nc, in_maps, list(range(N_CORES)))
    out = np.empty((B_SZ, L, D), np.float32)
    for c in range(N_CORES):
        b, h = c // 2, c % 2
        out[b, h * RPC:(h + 1) * RPC] = res.results[c]["out"].T
    return out
